# revision 1
# baseline (speedup 1.0000x reference)
"""HANConv Trainium2 kernel (8 NeuronCores, SPMD, full-I/O contract).

Strategy
--------
Destination-sharded, fully core-independent:
  * Each core owns 1/8 of destination nodes for BOTH relations
    (writes: author->paper, written: paper->author).
  * Edges are sorted by destination window (128 dst rows) on host. Per
    window, source rows are gathered from a bf16 copy of the raw source
    features via gpsimd.dma_gather (int16 indices => lo/hi table split),
    and segment-summed with one-hot matmuls accumulating in PSUM.
  * Aggregating RAW features (M = A_norm @ x) lets the relation transform
    and the semantic-score transform both become dense matmuls from M with
    host-folded weights (W_rel, W_rel @ W_sem), so no cross-core exchange
    of transformed features is ever needed.
  * Self path is computed from host-transposed x slices with folded
    weights (W_self, W_self @ W_sem) so no on-chip transpose is needed.
  * 2-candidate semantic softmax == sigmoid of score difference.
"""

import sys

sys.path.insert(0, "/opt/trn_rl_repo")

import numpy as np
import ml_dtypes

import concourse.bacc as bacc
import concourse.mybir as mybir
import concourse.tile as tile
from concourse.bass_utils import run_bass_kernel_spmd

P = 128
N = 50000
D = 256
HALF = 32768  # int16 gather index limit
NCORES = 8
NW_TOTAL = (N + P - 1) // P            # 391 destination windows
NWIN = (NW_TOTAL + NCORES - 1) // NCORES  # 49 windows per core
NW_ALLOC = NWIN * NCORES               # 392 (incl. 1 phantom window)
NPAD = NWIN * P                        # 6272 output rows per core

BF16 = ml_dtypes.bfloat16
F32 = np.float32

# (mps, tps, dps, sb, gbuf, oh) pool bufs
POOL_CFG = (2, 1, 1, 4, 4, 4)


# ---------------------------------------------------------------- host prep
def _prep_relation(row, col):
    """Sort edges by (dst window, src half); pad each group to 128 multiples.

    Returns idx16 [16, NW_ALLOC, 8*call], colf [P, NW_ALLOC, call],
    recip [P, NW_ALLOC], c_lo, c_hi.
    """
    E = row.shape[0]
    key = (col // P) * 2 + (row >= HALF)
    order = np.argsort(key, kind="stable")
    ks = key[order]
    rs = row[order].astype(np.int64)
    cs = col[order].astype(np.int64)

    counts = np.bincount(key, minlength=NW_TOTAL * 2)
    lo_cnt = counts[0::2]
    hi_cnt = counts[1::2]
    c_lo = max(1, int(-(-lo_cnt.max() // P)))
    c_hi = max(1, int(-(-hi_cnt.max() // P)))
    call = c_lo + c_hi

    grp_start = np.zeros(NW_TOTAL * 2 + 1, dtype=np.int64)
    np.cumsum(counts, out=grp_start[1:])
    rank = np.arange(E, dtype=np.int64) - grp_start[ks]
    w_of = ks // 2
    hi_of = ks % 2

    idx_flat = np.zeros(NW_ALLOC * call * P, dtype=np.int16)
    col_flat = np.full(NW_ALLOC * call * P, -1.0, dtype=F32)
    pos = w_of * (call * P) + hi_of * (c_lo * P) + rank
    idx_flat[pos] = (rs - HALF * hi_of).astype(np.int16)
    col_flat[pos] = (cs - w_of * P).astype(F32)

    idx_all = idx_flat.reshape(NW_ALLOC, call * P)
    # wrap for dma_gather: idx i of a gather block -> [i % 16, i // 16]
    lo_wr = idx_all[:, : c_lo * P].reshape(NW_ALLOC, c_lo * 8, 16).transpose(2, 0, 1)
    hi_wr = idx_all[:, c_lo * P:].reshape(NW_ALLOC, c_hi * 8, 16).transpose(2, 0, 1)
    idx16 = np.concatenate([lo_wr, hi_wr], axis=2)  # [16, NW_ALLOC, 8*call]

    colf = col_flat.reshape(NW_ALLOC, call, P).transpose(2, 0, 1)  # [P, NW, call]

    deg = np.bincount(col, minlength=NW_ALLOC * P).astype(F32)[: NW_ALLOC * P]
    recip = (1.0 / np.maximum(deg, 1.0)).reshape(NW_ALLOC, P).T  # [P, NW]
    return idx16, colf, recip, c_lo, c_hi


def _host_prep(inp):
    pr = {}
    pr["wr"] = _prep_relation(np.asarray(inp["row_writes"]), np.asarray(inp["col_writes"]))
    pr["wn"] = _prep_relation(np.asarray(inp["row_written"]), np.asarray(inp["col_written"]))

    xa = np.asarray(inp["x_author"], dtype=F32)
    xp = np.asarray(inp["x_paper"], dtype=F32)
    pr["xba"] = xa.astype(BF16)
    pr["xbp"] = xp.astype(BF16)

    # per-core transposed x slices (for the self path of the dst shard)
    xta, xtp = [], []
    for c in range(NCORES):
        r0, r1 = c * NPAD, min(N, (c + 1) * NPAD)
        sa = np.zeros((D, NPAD), dtype=BF16)
        sp = np.zeros((D, NPAD), dtype=BF16)
        sa[:, : r1 - r0] = xa[r0:r1].T
        sp[:, : r1 - r0] = xp[r0:r1].T
        xta.append(sa)
        xtp.append(sp)
    pr["xta"], pr["xtp"] = xta, xtp

    W_sem = np.asarray(inp["W_sem"], dtype=F32)
    b_sem = np.asarray(inp["b_sem"], dtype=F32)
    w_score = np.asarray(inp["w_score"], dtype=F32)

    def w(name):
        return np.asarray(inp[name], dtype=F32)

    pr["w_self_a"] = w("W_self_author").astype(BF16)
    pr["w_self_p"] = w("W_self_paper").astype(BF16)
    pr["wf_self_a"] = (w("W_self_author") @ W_sem).astype(BF16)
    pr["wf_self_p"] = (w("W_self_paper") @ W_sem).astype(BF16)
    pr["w_rel_wr"] = w("W_rel_writes").astype(BF16)
    pr["w_rel_wn"] = w("W_rel_written").astype(BF16)
    pr["wf_rel_wr"] = (w("W_rel_writes") @ W_sem).astype(BF16)
    pr["wf_rel_wn"] = (w("W_rel_written") @ W_sem).astype(BF16)

    rep = lambda v: np.tile(v.astype(F32), (P, 1))
    pr["b_self_a_rep"] = rep(w("b_self_author"))
    pr["b_self_p_rep"] = rep(w("b_self_paper"))
    pr["bf_self_a_rep"] = rep(w("b_self_author") @ W_sem + b_sem)
    pr["bf_self_p_rep"] = rep(w("b_self_paper") @ W_sem + b_sem)
    pr["bsem_rep"] = rep(b_sem)
    pr["w_rep"] = rep(w_score)

    pr["iota"] = np.tile(np.arange(P, dtype=F32), (P, 1)).astype(BF16)
    pr["ident"] = np.eye(P, dtype=F32).astype(BF16)
    return pr


# ---------------------------------------------------------------- program
def build_program(nwin, c_lo_wr, c_hi_wr, c_lo_wn, c_hi_wn):
    f32 = mybir.dt.float32
    bf16 = mybir.dt.bfloat16
    i16 = mybir.dt.int16
    AF = mybir.ActivationFunctionType
    OP = mybir.AluOpType

    call_wr = c_lo_wr + c_hi_wr
    call_wn = c_lo_wn + c_hi_wn
    npad = nwin * P

    nc = bacc.Bacc("TRN2", target_bir_lowering=False, debug=False)

    _mb, _tb, _db, _sb, _gb, _ob = POOL_CFG

    xba = nc.dram_tensor("xba", [N, D], bf16, kind="ExternalInput")
    xbp = nc.dram_tensor("xbp", [N, D], bf16, kind="ExternalInput")
    xta = nc.dram_tensor("xta", [D, npad], bf16, kind="ExternalInput")
    xtp = nc.dram_tensor("xtp", [D, npad], bf16, kind="ExternalInput")

    wnames = ["w_self_a", "wf_self_a", "w_self_p", "wf_self_p",
              "w_rel_wr", "wf_rel_wr", "w_rel_wn", "wf_rel_wn"]
    wdram = {n: nc.dram_tensor(n, [D, D], bf16, kind="ExternalInput") for n in wnames}
    bnames = ["b_self_a_rep", "bf_self_a_rep", "b_self_p_rep", "bf_self_p_rep",
              "bsem_rep", "w_rep"]
    bdram = {n: nc.dram_tensor(n, [P, D], f32, kind="ExternalInput") for n in bnames}
    iota_d = nc.dram_tensor("iota", [P, P], bf16, kind="ExternalInput")
    ident_d = nc.dram_tensor("ident", [P, P], bf16, kind="ExternalInput")

    idx_wr_d = nc.dram_tensor("idx_wr", [P, nwin * 8 * call_wr], i16, kind="ExternalInput")
    idx_wn_d = nc.dram_tensor("idx_wn", [P, nwin * 8 * call_wn], i16, kind="ExternalInput")
    colf_wr_d = nc.dram_tensor("colf_wr", [P, nwin * call_wr], bf16, kind="ExternalInput")
    colf_wn_d = nc.dram_tensor("colf_wn", [P, nwin * call_wn], bf16, kind="ExternalInput")
    recip_wr_d = nc.dram_tensor("recip_wr", [P, nwin], f32, kind="ExternalInput")
    recip_wn_d = nc.dram_tensor("recip_wn", [P, nwin], f32, kind="ExternalInput")

    oa = nc.dram_tensor("oa", [npad, D], f32, kind="ExternalOutput")
    op_ = nc.dram_tensor("op", [npad, D], f32, kind="ExternalOutput")

    with tile.TileContext(nc) as tc:
        with tc.tile_pool(name="const", bufs=1) as cpool, \
             tc.tile_pool(name="gbuf", bufs=_gb) as gpool, \
             tc.tile_pool(name="oh", bufs=_ob) as ohpool, \
             tc.tile_pool(name="sb", bufs=_sb) as sbpool, \
             tc.tile_pool(name="mps", bufs=_mb, space="PSUM") as mpool, \
             tc.tile_pool(name="tps", bufs=_tb, space="PSUM") as tpool, \
             tc.tile_pool(name="dps", bufs=_db, space="PSUM") as dpool:

            def load(dram, shape, dtype, tag):
                t = cpool.tile(shape, dtype, tag=tag)
                nc.sync.dma_start(t[:], dram)
                return t

            iota_t = load(iota_d[:], [P, P], bf16, "c_iota")
            ident_t = load(ident_d[:], [P, P], bf16, "c_ident")
            wt = {n: (load(wdram[n][0:P, :], [P, D], bf16, f"c_{n}0"),
                      load(wdram[n][P:D, :], [P, D], bf16, f"c_{n}1")) for n in wnames}
            bt = {n: load(bdram[n][:], [P, D], f32, f"c_{n}") for n in bnames}
            xta_t = (load(xta[0:P, :], [P, npad], bf16, "c_xta0"),
                     load(xta[P:D, :], [P, npad], bf16, "c_xta1"))
            xtp_t = (load(xtp[0:P, :], [P, npad], bf16, "c_xtp0"),
                     load(xtp[P:D, :], [P, npad], bf16, "c_xtp1"))
            idx_wr_t = load(idx_wr_d[:], [P, nwin * 8 * call_wr], i16, "c_idxwr")
            idx_wn_t = load(idx_wn_d[:], [P, nwin * 8 * call_wn], i16, "c_idxwn")
            colf_wr_t = load(colf_wr_d[:], [P, nwin * call_wr], bf16, "c_colfwr")
            colf_wn_t = load(colf_wn_d[:], [P, nwin * call_wn], bf16, "c_colfwn")
            recip_wr_t = load(recip_wr_d[:], [P, nwin], f32, "c_recipwr")
            recip_wn_t = load(recip_wn_d[:], [P, nwin], f32, "c_recipwn")

            rels = [
                dict(tag="wr", table=xba, idx=idx_wr_t, colf=colf_wr_t,
                     recip=recip_wr_t, c_lo=c_lo_wr, c_hi=c_hi_wr,
                     xt=xtp_t, w_self=wt["w_self_p"], wf_self=wt["wf_self_p"],
                     w_rel=wt["w_rel_wr"], wf_rel=wt["wf_rel_wr"],
                     b_self=bt["b_self_p_rep"], bf_self=bt["bf_self_p_rep"],
                     out=op_),
                dict(tag="wn", table=xbp, idx=idx_wn_t, colf=colf_wn_t,
                     recip=recip_wn_t, c_lo=c_lo_wn, c_hi=c_hi_wn,
                     xt=xta_t, w_self=wt["w_self_a"], wf_self=wt["wf_self_a"],
                     w_rel=wt["w_rel_wn"], wf_rel=wt["wf_rel_wn"],
                     b_self=bt["b_self_a_rep"], bf_self=bt["bf_self_a_rep"],
                     out=oa),
            ]

            def emit_window(w, r):
                c_lo, c_hi = r["c_lo"], r["c_hi"]
                call = c_lo + c_hi
                ic0 = w * 8 * call

                g_lo = gpool.tile([P, c_lo, D], bf16, tag="glo")
                nc.gpsimd.dma_gather(
                    g_lo[:], r["table"][:], r["idx"][:, ic0: ic0 + 8 * c_lo],
                    c_lo * P, c_lo * P, D, single_packet=False)
                g_hi = gpool.tile([P, c_hi, D], bf16, tag="ghi")
                nc.gpsimd.dma_gather(
                    g_hi[:], r["table"][HALF:, :],
                    r["idx"][:, ic0 + 8 * c_lo: ic0 + 8 * call],
                    c_hi * P, c_hi * P, D, single_packet=False)

                oh = ohpool.tile([P, call, P], bf16, tag="oh")
                nc.vector.tensor_tensor(
                    out=oh[:],
                    in0=r["colf"][:, w * call: (w + 1) * call, None].to_broadcast([P, call, P]),
                    in1=iota_t[:, None, :].to_broadcast([P, call, P]),
                    op=OP.is_equal)

                m_ps = mpool.tile([P, D], f32, tag="m")
                for k in range(call):
                    rhs = g_lo[:, k, :] if k < c_lo else g_hi[:, k - c_lo, :]
                    nc.tensor.matmul(out=m_ps[:], lhsT=oh[:, k, :], rhs=rhs,
                                     start=(k == 0), stop=(k == call - 1))

                m_sb = sbpool.tile([P, D], bf16, tag="m_sb")
                nc.vector.tensor_tensor(
                    out=m_sb[:], in0=m_ps[:],
                    in1=r["recip"][:, w: w + 1].to_broadcast([P, D]), op=OP.mult)

                mt = []
                for h2 in range(2):
                    t_ps = tpool.tile([P, P], bf16, tag="t")
                    nc.tensor.transpose(out=t_ps[:], in_=m_sb[:, h2 * P: (h2 + 1) * P],
                                        identity=ident_t[:])
                    mt_sb = sbpool.tile([P, P], bf16, tag=f"mt{h2}")
                    nc.vector.tensor_copy(out=mt_sb[:], in_=t_ps[:])
                    mt.append(mt_sb)

                def dense(lhsT0, lhsT1, wpair, ptag, pool=dpool):
                    ps = pool.tile([P, D], f32, tag=ptag)
                    nc.tensor.matmul(out=ps[:], lhsT=lhsT0, rhs=wpair[0][:],
                                     start=True, stop=False)
                    nc.tensor.matmul(out=ps[:], lhsT=lhsT1, rhs=wpair[1][:],
                                     start=False, stop=True)
                    return ps

                agg_ps = dense(mt[0][:], mt[1][:], r["w_rel"], "agg")
                sarg_ps = dense(mt[0][:], mt[1][:], r["wf_rel"], "sarg")
                xsl0 = r["xt"][0][:, w * P: (w + 1) * P]
                xsl1 = r["xt"][1][:, w * P: (w + 1) * P]
                h_ps = dense(xsl0, xsl1, r["w_self"], "h")
                sh_ps = dense(xsl0, xsl1, r["wf_self"], "sh")

                def score(ps, brep, stag):
                    targ = sbpool.tile([P, D], f32, tag=f"targ{stag}")
                    nc.vector.tensor_add(out=targ[:], in0=ps[:], in1=brep[:])
                    ttan = sbpool.tile([P, D], f32, tag=f"ttan{stag}")
                    nc.scalar.activation(out=ttan[:], in_=targ[:], func=AF.Tanh)
                    scr = sbpool.tile([P, D], f32, tag=f"scr{stag}")
                    nc.vector.tensor_mul(out=scr[:], in0=ttan[:], in1=bt["w_rep"][:])
                    s = sbpool.tile([P, 1], f32, tag=f"s{stag}")
                    nc.vector.tensor_reduce(out=s[:], in_=scr[:],
                                            axis=mybir.AxisListType.X,
                                            op=OP.add)
                    return s

                s_agg = score(sarg_ps, bt["bsem_rep"], "a")
                s_h = score(sh_ps, r["bf_self"], "h")

                h_sb = sbpool.tile([P, D], f32, tag="h_sb")
                nc.vector.tensor_add(out=h_sb[:], in0=h_ps[:], in1=r["b_self"][:])

                dsc = sbpool.tile([P, 1], f32, tag="dsc")
                nc.vector.tensor_sub(out=dsc[:], in0=s_h[:], in1=s_agg[:])
                a0 = sbpool.tile([P, 1], f32, tag="a0")
                nc.scalar.activation(out=a0[:], in_=dsc[:], func=AF.Sigmoid)

                diff = sbpool.tile([P, D], f32, tag="diff")
                nc.vector.tensor_sub(out=diff[:], in0=h_sb[:], in1=agg_ps[:])
                wd = sbpool.tile([P, D], f32, tag="wd")
                nc.vector.tensor_tensor(out=wd[:], in0=diff[:],
                                        in1=a0[:, 0:1].to_broadcast([P, D]),
                                        op=OP.mult)
                outt = sbpool.tile([P, D], f32, tag="outt")
                nc.vector.tensor_add(out=outt[:], in0=wd[:], in1=agg_ps[:])
                nc.sync.dma_start(r["out"][w * P: (w + 1) * P, :], outt[:])

            for w in range(nwin):
                for r in rels:
                    emit_window(w, r)

    nc.compile()
    return nc


# ---------------------------------------------------------------- driver
_PROG_CACHE = {}


def _get_program(key):
    if key not in _PROG_CACHE:
        _PROG_CACHE[key] = build_program(*key)
    return _PROG_CACHE[key]


def _make_in_maps(pr):
    shared = dict(
        xba=pr["xba"], xbp=pr["xbp"],
        iota=pr["iota"], ident=pr["ident"],
        bsem_rep=pr["bsem_rep"], w_rep=pr["w_rep"],
        b_self_a_rep=pr["b_self_a_rep"], b_self_p_rep=pr["b_self_p_rep"],
        bf_self_a_rep=pr["bf_self_a_rep"], bf_self_p_rep=pr["bf_self_p_rep"],
        w_self_a=pr["w_self_a"], w_self_p=pr["w_self_p"],
        wf_self_a=pr["wf_self_a"], wf_self_p=pr["wf_self_p"],
        w_rel_wr=pr["w_rel_wr"], w_rel_wn=pr["w_rel_wn"],
        wf_rel_wr=pr["wf_rel_wr"], wf_rel_wn=pr["wf_rel_wn"],
    )
    idx_wr, colf_wr, recip_wr, _, _ = pr["wr"]
    idx_wn, colf_wn, recip_wn, _, _ = pr["wn"]
    in_maps = []
    for c in range(NCORES):
        w0, w1 = c * NWIN, (c + 1) * NWIN
        m = dict(shared)
        m["xta"] = pr["xta"][c]
        m["xtp"] = pr["xtp"][c]
        m["idx_wr"] = np.ascontiguousarray(
            np.tile(idx_wr[:, w0:w1].reshape(16, -1), (8, 1)))
        m["idx_wn"] = np.ascontiguousarray(
            np.tile(idx_wn[:, w0:w1].reshape(16, -1), (8, 1)))
        m["colf_wr"] = np.ascontiguousarray(colf_wr[:, w0:w1].reshape(P, -1)).astype(BF16)
        m["colf_wn"] = np.ascontiguousarray(colf_wn[:, w0:w1].reshape(P, -1)).astype(BF16)
        m["recip_wr"] = np.ascontiguousarray(recip_wr[:, w0:w1])
        m["recip_wn"] = np.ascontiguousarray(recip_wn[:, w0:w1])
        in_maps.append(m)
    return in_maps


def run(trace=False, tmpdir=None, **inputs):
    pr = _host_prep(inputs)
    _, _, _, c_lo_wr, c_hi_wr = pr["wr"]
    _, _, _, c_lo_wn, c_hi_wn = pr["wn"]
    nc = _get_program((NWIN, c_lo_wr, c_hi_wr, c_lo_wn, c_hi_wn))
    in_maps = _make_in_maps(pr)
    res = run_bass_kernel_spmd(nc, in_maps, list(range(NCORES)),
                               trace=trace, tmpdir=tmpdir)
    oa = np.empty((N, D), dtype=F32)
    op = np.empty((N, D), dtype=F32)
    for c in range(NCORES):
        r0, r1 = c * NPAD, min(N, (c + 1) * NPAD)
        oa[r0:r1] = res.results[c]["oa"][: r1 - r0]
        op[r0:r1] = res.results[c]["op"][: r1 - r0]
    return (oa, op), res


def kernel(**inputs):
    (oa, op), _ = run(trace=False, **inputs)
    return (oa, op)



# revision 3
# speedup vs baseline: 1.0942x; 1.0942x over previous
"""HANConv Trainium2 kernel (8 NeuronCores, SPMD, full-I/O contract).

Strategy
--------
Destination-sharded, fully core-independent, zero on-device gather:
  * Destination nodes of each type are PERMUTED into 392 balanced windows
    (128 nodes each) so every window receives <= 2048 edges => exactly
    `call` 128-edge chunks per window, identical across cores (SPMD).
  * Source features for each relation are PRE-GATHERED on host into edge
    order (sorted by destination window) and streamed to each core as
    large contiguous HWDGE DMAs (1 MB per window) -- this replaces the
    gpsimd dma_gather, which costs ~28 us fixed per call on HW.
  * Per window, segment-sum is one-hot matmuls accumulating in PSUM
    (aggregating RAW features; relation + semantic-score transforms fold
    into dense matmuls afterwards with host-folded [W | W@W_sem] pairs).
  * Self path from host-transposed permuted x slices.
  * 2-candidate semantic softmax == sigmoid of score difference.
"""

import heapq
import sys

sys.path.insert(0, "/opt/trn_rl_repo")

import numpy as np
import ml_dtypes

import concourse.bacc as bacc
import concourse.mybir as mybir
import concourse.tile as tile
from concourse.bass_utils import run_bass_kernel_spmd

P = 128
N = 50000
D = 256
NCORES = 8
NWIN = 49                 # windows per core
NW = NWIN * NCORES        # 392 windows total
NPAD = NWIN * P           # 6272 rows per core

BF16 = ml_dtypes.bfloat16
F32 = np.float32


# ---------------------------------------------------------------- host prep
def _balance_windows(deg):
    """LPT-pack destination nodes into NW windows of <=P nodes, balancing
    edge counts. Returns node_at [NW, P] (node id or -1) and call."""
    order = np.argsort(-deg, kind="stable")
    heap = [(0, w, 0) for w in range(NW)]
    heapq.heapify(heap)
    node_at = np.full((NW, P), -1, dtype=np.int64)
    sums = np.zeros(NW, dtype=np.int64)
    for n in order:
        s, w, c = heapq.heappop(heap)
        node_at[w, c] = n
        s += int(deg[n])
        c += 1
        sums[w] = s
        if c < P:
            heapq.heappush(heap, (s, w, c))
    call = max(1, int(-(-sums.max() // P)))
    return node_at, call


def _prep_relation(x_src_b, row, col):
    """Sort edges by balanced dst window; pre-gather source features.

    Returns dict with g [NW, P, call*D] bf16, colf [P, NW, call] bf16,
    recip [P, NW] f32, node_at [NW, P], call.
    """
    E = row.shape[0]
    deg = np.bincount(col, minlength=N)
    node_at, call = _balance_windows(deg)

    # node -> (window, slot)
    win_of = np.empty(N, dtype=np.int64)
    slot_of = np.empty(N, dtype=np.int64)
    flat = node_at.ravel()
    valid = flat >= 0
    pos = np.arange(NW * P, dtype=np.int64)[valid]
    win_of[flat[valid]] = pos // P
    slot_of[flat[valid]] = pos % P

    ew = win_of[col]                       # edge -> window
    order = np.argsort(ew, kind="stable")
    ew_s = ew[order]
    rows_s = row[order].astype(np.int64)
    slots_s = slot_of[col[order]]

    cnt = np.bincount(ew_s, minlength=NW)
    assert cnt.max() <= call * P, (cnt.max(), call * P)
    starts = np.zeros(NW + 1, dtype=np.int64)
    np.cumsum(cnt, out=starts[1:])
    rank = np.arange(E, dtype=np.int64) - starts[ew_s]
    dstpos = ew_s * (call * P) + rank

    rows_pad = np.zeros(NW * call * P, dtype=np.int64)
    slot_pad = np.full(NW * call * P, -1.0, dtype=F32)
    rows_pad[dstpos] = rows_s
    slot_pad[dstpos] = slots_s.astype(F32)

    # pre-gather + per-window [P, call*D] layout (partition = edge-in-chunk)
    g = x_src_b[rows_pad]                                    # [NW*call*P, D]
    g = g.reshape(NW, call, P, D).transpose(0, 2, 1, 3)      # [NW, P, call, D]

    colf = slot_pad.reshape(NW, call, P).transpose(2, 0, 1)  # [P, NW, call]

    recip = np.ones((NW, P), dtype=F32)
    recip[node_at >= 0] = 1.0 / np.maximum(deg[node_at[node_at >= 0]], 1.0)

    return dict(g=g, colf=colf.astype(BF16), recip=recip.T.copy(),
                node_at=node_at, call=call)


def _host_prep(inp):
    xa = np.asarray(inp["x_author"], dtype=F32)
    xp = np.asarray(inp["x_paper"], dtype=F32)
    xa_b = xa.astype(BF16)
    xp_b = xp.astype(BF16)

    pr = {}
    # writes: author -> paper (dst type paper); written: paper -> author
    pr["wr"] = _prep_relation(xa_b, np.asarray(inp["row_writes"]),
                              np.asarray(inp["col_writes"]))
    pr["wn"] = _prep_relation(xp_b, np.asarray(inp["row_written"]),
                              np.asarray(inp["col_written"]))

    # permuted self-path features (dst of wr = paper, dst of wn = author)
    def xperm(x, node_at):
        out = np.zeros((NW * P, D), dtype=BF16)
        flat = node_at.ravel()
        v = flat >= 0
        out[v] = x[flat[v]]
        return out

    xp_perm = xperm(xp_b, pr["wr"]["node_at"])
    xa_perm = xperm(xa_b, pr["wn"]["node_at"])
    pr["xtp"] = [np.ascontiguousarray(xp_perm[c * NPAD:(c + 1) * NPAD].T)
                 for c in range(NCORES)]
    pr["xta"] = [np.ascontiguousarray(xa_perm[c * NPAD:(c + 1) * NPAD].T)
                 for c in range(NCORES)]

    W_sem = np.asarray(inp["W_sem"], dtype=F32)
    b_sem = np.asarray(inp["b_sem"], dtype=F32)
    w_score = np.asarray(inp["w_score"], dtype=F32)

    def w(name):
        return np.asarray(inp[name], dtype=F32)

    def pair(W):
        return np.ascontiguousarray(
            np.concatenate([W, W @ W_sem], axis=1)).astype(BF16)

    pr["wp_self_p"] = pair(w("W_self_paper"))
    pr["wp_self_a"] = pair(w("W_self_author"))
    pr["wp_rel_wr"] = pair(w("W_rel_writes"))
    pr["wp_rel_wn"] = pair(w("W_rel_written"))

    rep = lambda v: np.tile(v.astype(F32), (P, 1))
    pr["b_self_p_rep"] = rep(w("b_self_paper"))
    pr["b_self_a_rep"] = rep(w("b_self_author"))
    pr["bf_self_p_rep"] = rep(w("b_self_paper") @ W_sem + b_sem)
    pr["bf_self_a_rep"] = rep(w("b_self_author") @ W_sem + b_sem)
    pr["bsem_rep"] = rep(b_sem)
    pr["w_rep"] = rep(w_score)

    pr["iota"] = np.tile(np.arange(P, dtype=F32), (P, 1)).astype(BF16)
    pr["ident"] = np.eye(P, dtype=F32).astype(BF16)
    return pr


# ---------------------------------------------------------------- program
def build_program(call_wr, call_wn):
    f32 = mybir.dt.float32
    bf16 = mybir.dt.bfloat16
    AF = mybir.ActivationFunctionType
    OP = mybir.AluOpType

    nc = bacc.Bacc("TRN2", target_bir_lowering=False, debug=False)

    g_wr_d = nc.dram_tensor("g_wr", [NWIN * P, call_wr * D], bf16,
                            kind="ExternalInput")
    g_wn_d = nc.dram_tensor("g_wn", [NWIN * P, call_wn * D], bf16,
                            kind="ExternalInput")
    xtp_d = nc.dram_tensor("xtp", [D, NPAD], bf16, kind="ExternalInput")
    xta_d = nc.dram_tensor("xta", [D, NPAD], bf16, kind="ExternalInput")

    wnames = ["wp_self_p", "wp_self_a", "wp_rel_wr", "wp_rel_wn"]
    wdram = {n: nc.dram_tensor(n, [D, 2 * D], bf16, kind="ExternalInput")
             for n in wnames}
    bnames = ["b_self_p_rep", "b_self_a_rep", "bf_self_p_rep",
              "bf_self_a_rep", "bsem_rep", "w_rep"]
    bdram = {n: nc.dram_tensor(n, [P, D], f32, kind="ExternalInput")
             for n in bnames}
    iota_d = nc.dram_tensor("iota", [P, P], bf16, kind="ExternalInput")
    ident_d = nc.dram_tensor("ident", [P, P], bf16, kind="ExternalInput")

    colf_wr_d = nc.dram_tensor("colf_wr", [P, NWIN * call_wr], bf16,
                               kind="ExternalInput")
    colf_wn_d = nc.dram_tensor("colf_wn", [P, NWIN * call_wn], bf16,
                               kind="ExternalInput")
    recip_wr_d = nc.dram_tensor("recip_wr", [P, NWIN], f32, kind="ExternalInput")
    recip_wn_d = nc.dram_tensor("recip_wn", [P, NWIN], f32, kind="ExternalInput")

    oa = nc.dram_tensor("oa", [NPAD, D], f32, kind="ExternalOutput")
    op_ = nc.dram_tensor("op", [NPAD, D], f32, kind="ExternalOutput")

    with tile.TileContext(nc) as tc:
        with tc.tile_pool(name="const", bufs=1) as cpool, \
             tc.tile_pool(name="g", bufs=3) as gpool, \
             tc.tile_pool(name="oh", bufs=3) as ohpool, \
             tc.tile_pool(name="sb", bufs=4) as sbpool, \
             tc.tile_pool(name="mps", bufs=2, space="PSUM") as mpool, \
             tc.tile_pool(name="tps", bufs=2, space="PSUM") as tpool, \
             tc.tile_pool(name="dps", bufs=2, space="PSUM") as dpool:

            def load(dram, shape, dtype, tag):
                t = cpool.tile(shape, dtype, tag=tag)
                nc.sync.dma_start(t[:], dram)
                return t

            iota_t = load(iota_d[:], [P, P], bf16, "c_iota")
            ident_t = load(ident_d[:], [P, P], bf16, "c_ident")
            wt = {n: (load(wdram[n][0:P, :], [P, 2 * D], bf16, f"c_{n}0"),
                      load(wdram[n][P:D, :], [P, 2 * D], bf16, f"c_{n}1"))
                  for n in wnames}
            bt = {n: load(bdram[n][:], [P, D], f32, f"c_{n}") for n in bnames}
            xtp_t = (load(xtp_d[0:P, :], [P, NPAD], bf16, "c_xtp0"),
                     load(xtp_d[P:D, :], [P, NPAD], bf16, "c_xtp1"))
            xta_t = (load(xta_d[0:P, :], [P, NPAD], bf16, "c_xta0"),
                     load(xta_d[P:D, :], [P, NPAD], bf16, "c_xta1"))
            colf_wr_t = load(colf_wr_d[:], [P, NWIN * call_wr], bf16, "c_colfwr")
            colf_wn_t = load(colf_wn_d[:], [P, NWIN * call_wn], bf16, "c_colfwn")
            recip_wr_t = load(recip_wr_d[:], [P, NWIN], f32, "c_recipwr")
            recip_wn_t = load(recip_wn_d[:], [P, NWIN], f32, "c_recipwn")

            rels = [
                dict(tag="wr", gdram=g_wr_d, call=call_wr, colf=colf_wr_t,
                     recip=recip_wr_t, xt=xtp_t, wp_self=wt["wp_self_p"],
                     wp_rel=wt["wp_rel_wr"], b_self=bt["b_self_p_rep"],
                     bf_self=bt["bf_self_p_rep"], out=op_),
                dict(tag="wn", gdram=g_wn_d, call=call_wn, colf=colf_wn_t,
                     recip=recip_wn_t, xt=xta_t, wp_self=wt["wp_self_a"],
                     wp_rel=wt["wp_rel_wn"], b_self=bt["b_self_a_rep"],
                     bf_self=bt["bf_self_a_rep"], out=oa),
            ]

            def emit_window(w, r):
                call = r["call"]
                g = gpool.tile([P, call * D], bf16, tag="g")
                nc.sync.dma_start(g[:], r["gdram"][w * P:(w + 1) * P, :])

                oh = ohpool.tile([P, call, P], bf16, tag="oh")
                nc.vector.tensor_tensor(
                    out=oh[:],
                    in0=r["colf"][:, w * call:(w + 1) * call, None]
                        .to_broadcast([P, call, P]),
                    in1=iota_t[:, None, :].to_broadcast([P, call, P]),
                    op=OP.is_equal)

                m_ps = mpool.tile([P, D], f32, tag="m")
                for k in range(call):
                    nc.tensor.matmul(out=m_ps[:], lhsT=oh[:, k, :],
                                     rhs=g[:, k * D:(k + 1) * D],
                                     start=(k == 0), stop=(k == call - 1))

                m_sb = sbpool.tile([P, D], bf16, tag="m_sb")
                nc.vector.tensor_tensor(
                    out=m_sb[:], in0=m_ps[:],
                    in1=r["recip"][:, w:w + 1].to_broadcast([P, D]), op=OP.mult)

                mt = []
                for h2 in range(2):
                    t_ps = tpool.tile([P, P], bf16, tag="t")
                    nc.tensor.transpose(out=t_ps[:],
                                        in_=m_sb[:, h2 * P:(h2 + 1) * P],
                                        identity=ident_t[:])
                    mt_sb = sbpool.tile([P, P], bf16, tag=f"mt{h2}")
                    nc.vector.tensor_copy(out=mt_sb[:], in_=t_ps[:])
                    mt.append(mt_sb)

                rel_ps = dpool.tile([P, 2 * D], f32, tag="rel")
                nc.tensor.matmul(out=rel_ps[:], lhsT=mt[0][:],
                                 rhs=r["wp_rel"][0][:], start=True, stop=False)
                nc.tensor.matmul(out=rel_ps[:], lhsT=mt[1][:],
                                 rhs=r["wp_rel"][1][:], start=False, stop=True)

                self_ps = dpool.tile([P, 2 * D], f32, tag="self")
                xsl0 = r["xt"][0][:, w * P:(w + 1) * P]
                xsl1 = r["xt"][1][:, w * P:(w + 1) * P]
                nc.tensor.matmul(out=self_ps[:], lhsT=xsl0,
                                 rhs=r["wp_self"][0][:], start=True, stop=False)
                nc.tensor.matmul(out=self_ps[:], lhsT=xsl1,
                                 rhs=r["wp_self"][1][:], start=False, stop=True)

                def score(zslice, brep, stag):
                    targ = sbpool.tile([P, D], f32, tag=f"targ{stag}")
                    nc.vector.tensor_add(out=targ[:], in0=zslice, in1=brep[:])
                    ttan = sbpool.tile([P, D], f32, tag=f"ttan{stag}")
                    nc.scalar.activation(out=ttan[:], in_=targ[:], func=AF.Tanh)
                    scr = sbpool.tile([P, D], f32, tag=f"scr{stag}")
                    nc.vector.tensor_mul(out=scr[:], in0=ttan[:],
                                         in1=bt["w_rep"][:])
                    s = sbpool.tile([P, 1], f32, tag=f"s{stag}")
                    nc.vector.tensor_reduce(out=s[:], in_=scr[:],
                                            axis=mybir.AxisListType.X, op=OP.add)
                    return s

                s_agg = score(rel_ps[:, D:2 * D], bt["bsem_rep"], "a")
                s_h = score(self_ps[:, D:2 * D], r["bf_self"], "h")

                h_sb = sbpool.tile([P, D], f32, tag="h_sb")
                nc.vector.tensor_add(out=h_sb[:], in0=self_ps[:, 0:D],
                                     in1=r["b_self"][:])

                dsc = sbpool.tile([P, 1], f32, tag="dsc")
                nc.vector.tensor_sub(out=dsc[:], in0=s_h[:], in1=s_agg[:])
                a0 = sbpool.tile([P, 1], f32, tag="a0")
                nc.scalar.activation(out=a0[:], in_=dsc[:], func=AF.Sigmoid)

                diff = sbpool.tile([P, D], f32, tag="diff")
                nc.vector.tensor_sub(out=diff[:], in0=h_sb[:],
                                     in1=rel_ps[:, 0:D])
                wd = sbpool.tile([P, D], f32, tag="wd")
                nc.vector.tensor_tensor(out=wd[:], in0=diff[:],
                                        in1=a0[:, 0:1].to_broadcast([P, D]),
                                        op=OP.mult)
                outt = sbpool.tile([P, D], f32, tag="outt")
                nc.vector.tensor_add(out=outt[:], in0=wd[:],
                                     in1=rel_ps[:, 0:D])
                nc.sync.dma_start(r["out"][w * P:(w + 1) * P, :], outt[:])

            for w in range(NWIN):
                for r in rels:
                    emit_window(w, r)

    nc.compile()
    return nc


# ---------------------------------------------------------------- driver
_PROG_CACHE = {}


def _get_program(key):
    if key not in _PROG_CACHE:
        _PROG_CACHE[key] = build_program(*key)
    return _PROG_CACHE[key]


def _make_in_maps(pr):
    shared = dict(
        iota=pr["iota"], ident=pr["ident"],
        bsem_rep=pr["bsem_rep"], w_rep=pr["w_rep"],
        b_self_p_rep=pr["b_self_p_rep"], b_self_a_rep=pr["b_self_a_rep"],
        bf_self_p_rep=pr["bf_self_p_rep"], bf_self_a_rep=pr["bf_self_a_rep"],
        wp_self_p=pr["wp_self_p"], wp_self_a=pr["wp_self_a"],
        wp_rel_wr=pr["wp_rel_wr"], wp_rel_wn=pr["wp_rel_wn"],
    )
    wr, wn = pr["wr"], pr["wn"]
    in_maps = []
    for c in range(NCORES):
        w0, w1 = c * NWIN, (c + 1) * NWIN
        m = dict(shared)
        m["g_wr"] = np.ascontiguousarray(
            wr["g"][w0:w1]).reshape(NWIN * P, wr["call"] * D)
        m["g_wn"] = np.ascontiguousarray(
            wn["g"][w0:w1]).reshape(NWIN * P, wn["call"] * D)
        m["xtp"] = pr["xtp"][c]
        m["xta"] = pr["xta"][c]
        m["colf_wr"] = np.ascontiguousarray(
            wr["colf"][:, w0:w1].reshape(P, -1))
        m["colf_wn"] = np.ascontiguousarray(
            wn["colf"][:, w0:w1].reshape(P, -1))
        m["recip_wr"] = np.ascontiguousarray(wr["recip"][:, w0:w1])
        m["recip_wn"] = np.ascontiguousarray(wn["recip"][:, w0:w1])
        in_maps.append(m)
    return in_maps


def _unpermute(res_list, key, pr):
    node_at = pr[key]["node_at"]          # [NW, P]
    cat = np.concatenate([r for r in res_list], axis=0)  # [NW*P, D]
    out = np.empty((N, D), dtype=F32)
    flat = node_at.ravel()
    v = flat >= 0
    out[flat[v]] = cat[v]
    return out


def run(trace=False, tmpdir=None, **inputs):
    pr = _host_prep(inputs)
    nc = _get_program((pr["wr"]["call"], pr["wn"]["call"]))
    in_maps = _make_in_maps(pr)
    res = run_bass_kernel_spmd(nc, in_maps, list(range(NCORES)),
                               trace=trace, tmpdir=tmpdir)
    op = _unpermute([res.results[c]["op"] for c in range(NCORES)], "wr", pr)
    oa = _unpermute([res.results[c]["oa"] for c in range(NCORES)], "wn", pr)
    return (oa, op), res


def kernel(**inputs):
    (oa, op), _ = run(trace=False, **inputs)
    return (oa, op)


# revision 10
# speedup vs baseline: 1.2225x; 1.1172x over previous
"""HANConv Trainium2 kernel (8 NeuronCores, SPMD, full-I/O contract).

Strategy
--------
Destination-sharded, fully core-independent, zero on-device gather:
  * Destination nodes of each type are PERMUTED into 392 balanced windows
    (128 nodes each) so every window receives <= 2048 edges => exactly
    `call` 128-edge chunks per window, identical across cores (SPMD).
  * Source features for each relation are PRE-GATHERED on host into edge
    order (sorted by destination window) and streamed to each core as
    large contiguous HWDGE DMAs (1 MB per window) -- this replaces the
    gpsimd dma_gather, which costs ~28 us fixed per call on HW.
  * Per window, segment-sum is one-hot matmuls accumulating in PSUM
    (aggregating RAW features; relation + semantic-score transforms fold
    into dense matmuls afterwards with host-folded [W | W@W_sem] pairs).
  * Self path from host-transposed permuted x slices.
  * 2-candidate semantic softmax == sigmoid of score difference.
"""

import heapq
import sys

sys.path.insert(0, "/opt/trn_rl_repo")

import numpy as np
import ml_dtypes

import concourse.bacc as bacc
import concourse.mybir as mybir
import concourse.tile as tile
from concourse.bass_utils import run_bass_kernel_spmd

P = 128
N = 50000
D = 256
NCORES = 8
NWIN = 49                 # windows per core
NW = NWIN * NCORES        # 392 windows total
NPAD = NWIN * P           # 6272 rows per core

BF16 = ml_dtypes.bfloat16
FP8 = ml_dtypes.float8_e4m3
F32 = np.float32


# ---------------------------------------------------------------- host prep
def _balance_windows(deg):
    """LPT-pack destination nodes into NW windows of <=P nodes, balancing
    edge counts. Returns node_at [NW, P] (node id or -1) and call."""
    order = np.argsort(-deg, kind="stable")
    heap = [(0, w, 0) for w in range(NW)]
    heapq.heapify(heap)
    node_at = np.full((NW, P), -1, dtype=np.int64)
    sums = np.zeros(NW, dtype=np.int64)
    for n in order:
        s, w, c = heapq.heappop(heap)
        node_at[w, c] = n
        s += int(deg[n])
        c += 1
        sums[w] = s
        if c < P:
            heapq.heappush(heap, (s, w, c))
    call = max(1, int(-(-sums.max() // P)))
    return node_at, call


def _prep_relation(x_src_b, row, col):
    """Sort edges by balanced dst window; pre-gather source features.

    Returns dict with g [NW, P, call*D] bf16, colf [P, NW, call] bf16,
    recip [P, NW] f32, node_at [NW, P], call.
    """
    E = row.shape[0]
    deg = np.bincount(col, minlength=N)
    node_at, call = _balance_windows(deg)

    # node -> (window, slot)
    win_of = np.empty(N, dtype=np.int64)
    slot_of = np.empty(N, dtype=np.int64)
    flat = node_at.ravel()
    valid = flat >= 0
    pos = np.arange(NW * P, dtype=np.int64)[valid]
    win_of[flat[valid]] = pos // P
    slot_of[flat[valid]] = pos % P

    ew = win_of[col]                       # edge -> window
    order = np.argsort(ew, kind="stable")
    ew_s = ew[order]
    rows_s = row[order].astype(np.int64)
    slots_s = slot_of[col[order]]

    cnt = np.bincount(ew_s, minlength=NW)
    assert cnt.max() <= call * P, (cnt.max(), call * P)
    starts = np.zeros(NW + 1, dtype=np.int64)
    np.cumsum(cnt, out=starts[1:])
    rank = np.arange(E, dtype=np.int64) - starts[ew_s]
    dstpos = ew_s * (call * P) + rank

    rows_pad = np.zeros(NW * call * P, dtype=np.int64)
    slot_pad = np.full(NW * call * P, -1.0, dtype=F32)
    rows_pad[dstpos] = rows_s
    slot_pad[dstpos] = slots_s.astype(F32)

    # pre-gather + per-window [P, call*D] layout (partition = edge-in-chunk)
    g = x_src_b[rows_pad].astype(FP8)                        # [NW*call*P, D]
    g = g.reshape(NW, call, P, D).transpose(0, 2, 1, 3)      # [NW, P, call, D]

    colf = slot_pad.reshape(NW, call, P).transpose(2, 0, 1)  # [P, NW, call]

    recip = np.ones((NW, P), dtype=F32)
    recip[node_at >= 0] = 1.0 / np.maximum(deg[node_at[node_at >= 0]], 1.0)

    return dict(g=g, colf=colf.astype(BF16), recip=recip.T.copy(),
                node_at=node_at, call=call)


def _host_prep(inp):
    xa = np.asarray(inp["x_author"], dtype=F32)
    xp = np.asarray(inp["x_paper"], dtype=F32)
    xa_b = xa.astype(BF16)
    xp_b = xp.astype(BF16)

    pr = {}
    # writes: author -> paper (dst type paper); written: paper -> author
    pr["wr"] = _prep_relation(xa_b, np.asarray(inp["row_writes"]),
                              np.asarray(inp["col_writes"]))
    pr["wn"] = _prep_relation(xp_b, np.asarray(inp["row_written"]),
                              np.asarray(inp["col_written"]))

    # permuted self-path features (dst of wr = paper, dst of wn = author)
    def xperm(x, node_at):
        out = np.zeros((NW * P, D), dtype=BF16)
        flat = node_at.ravel()
        v = flat >= 0
        out[v] = x[flat[v]]
        return out

    xp_perm = xperm(xp_b, pr["wr"]["node_at"])
    xa_perm = xperm(xa_b, pr["wn"]["node_at"])
    pr["xtp"] = [np.ascontiguousarray(xp_perm[c * NPAD:(c + 1) * NPAD].T)
                 for c in range(NCORES)]
    pr["xta"] = [np.ascontiguousarray(xa_perm[c * NPAD:(c + 1) * NPAD].T)
                 for c in range(NCORES)]

    W_sem = np.asarray(inp["W_sem"], dtype=F32)
    b_sem = np.asarray(inp["b_sem"], dtype=F32)
    w_score = np.asarray(inp["w_score"], dtype=F32)

    def w(name):
        return np.asarray(inp[name], dtype=F32)

    def pair(W):
        return np.ascontiguousarray(
            np.concatenate([W, W @ W_sem], axis=1)).astype(BF16)

    pr["wp_self_p"] = pair(w("W_self_paper"))
    pr["wp_self_a"] = pair(w("W_self_author"))
    pr["wp_rel_wr"] = pair(w("W_rel_writes"))
    pr["wp_rel_wn"] = pair(w("W_rel_written"))

    rep = lambda v: np.tile(v.astype(F32), (P, 1))
    pr["b_self_p_rep"] = rep(w("b_self_paper"))
    pr["b_self_a_rep"] = rep(w("b_self_author"))
    pr["bf_self_p_rep"] = rep(w("b_self_paper") @ W_sem + b_sem)
    pr["bf_self_a_rep"] = rep(w("b_self_author") @ W_sem + b_sem)
    pr["bsem_rep"] = rep(b_sem)
    pr["w_rep"] = rep(w_score)

    pr["iota"] = np.tile(np.arange(P, dtype=F32), (P, 1)).astype(BF16)
    pr["ident"] = np.eye(P, dtype=F32).astype(BF16)
    return pr


# ---------------------------------------------------------------- program
def build_program(call_wr, call_wn):
    f32 = mybir.dt.float32
    bf16 = mybir.dt.bfloat16
    fp8 = mybir.dt.float8e4
    AF = mybir.ActivationFunctionType
    OP = mybir.AluOpType

    nc = bacc.Bacc("TRN2", target_bir_lowering=False, debug=False)

    g_wr_d = nc.dram_tensor("g_wr", [NWIN * P, call_wr * D], fp8,
                            kind="ExternalInput")
    g_wn_d = nc.dram_tensor("g_wn", [NWIN * P, call_wn * D], fp8,
                            kind="ExternalInput")
    xtp_d = nc.dram_tensor("xtp", [D, NPAD], bf16, kind="ExternalInput")
    xta_d = nc.dram_tensor("xta", [D, NPAD], bf16, kind="ExternalInput")

    wnames = ["wp_self_p", "wp_self_a", "wp_rel_wr", "wp_rel_wn"]
    wdram = {n: nc.dram_tensor(n, [D, 2 * D], bf16, kind="ExternalInput")
             for n in wnames}
    bnames = ["b_self_p_rep", "b_self_a_rep", "bf_self_p_rep",
              "bf_self_a_rep", "bsem_rep", "w_rep"]
    bdram = {n: nc.dram_tensor(n, [P, D], f32, kind="ExternalInput")
             for n in bnames}
    iota_d = nc.dram_tensor("iota", [P, P], bf16, kind="ExternalInput")
    ident_d = nc.dram_tensor("ident", [P, P], bf16, kind="ExternalInput")

    colf_wr_d = nc.dram_tensor("colf_wr", [P, NWIN * call_wr], bf16,
                               kind="ExternalInput")
    colf_wn_d = nc.dram_tensor("colf_wn", [P, NWIN * call_wn], bf16,
                               kind="ExternalInput")
    recip_wr_d = nc.dram_tensor("recip_wr", [P, NWIN], f32, kind="ExternalInput")
    recip_wn_d = nc.dram_tensor("recip_wn", [P, NWIN], f32, kind="ExternalInput")

    oa = nc.dram_tensor("oa", [NPAD, D], bf16, kind="ExternalOutput")
    op_ = nc.dram_tensor("op", [NPAD, D], bf16, kind="ExternalOutput")

    with tile.TileContext(nc) as tc:
        with tc.tile_pool(name="const", bufs=1) as cpool, \
             tc.tile_pool(name="g", bufs=3) as gpool, \
             tc.tile_pool(name="oh", bufs=3) as ohpool, \
             tc.tile_pool(name="sb", bufs=4) as sbpool, \
             tc.tile_pool(name="mps", bufs=2, space="PSUM") as mpool, \
             tc.tile_pool(name="tps", bufs=2, space="PSUM") as tpool, \
             tc.tile_pool(name="dps", bufs=2, space="PSUM") as dpool:

            def load(dram, shape, dtype, tag):
                t = cpool.tile(shape, dtype, tag=tag)
                nc.sync.dma_start(t[:], dram)
                return t

            iota_t = load(iota_d[:], [P, P], bf16, "c_iota")
            ident_t = load(ident_d[:], [P, P], bf16, "c_ident")
            wt = {n: (load(wdram[n][0:P, :], [P, 2 * D], bf16, f"c_{n}0"),
                      load(wdram[n][P:D, :], [P, 2 * D], bf16, f"c_{n}1"))
                  for n in wnames}
            bt = {n: load(bdram[n][:], [P, D], f32, f"c_{n}") for n in bnames}
            xtp_t = (load(xtp_d[0:P, :], [P, NPAD], bf16, "c_xtp0"),
                     load(xtp_d[P:D, :], [P, NPAD], bf16, "c_xtp1"))
            xta_t = (load(xta_d[0:P, :], [P, NPAD], bf16, "c_xta0"),
                     load(xta_d[P:D, :], [P, NPAD], bf16, "c_xta1"))
            colf_wr_t = load(colf_wr_d[:], [P, NWIN * call_wr], bf16, "c_colfwr")
            colf_wn_t = load(colf_wn_d[:], [P, NWIN * call_wn], bf16, "c_colfwn")
            recip_wr_t = load(recip_wr_d[:], [P, NWIN], f32, "c_recipwr")
            recip_wn_t = load(recip_wn_d[:], [P, NWIN], f32, "c_recipwn")

            rels = [
                dict(tag="wr", gdram=g_wr_d, call=call_wr, colf=colf_wr_t,
                     recip=recip_wr_t, xt=xtp_t, wp_self=wt["wp_self_p"],
                     wp_rel=wt["wp_rel_wr"], b_self=bt["b_self_p_rep"],
                     bf_self=bt["bf_self_p_rep"], out=op_),
                dict(tag="wn", gdram=g_wn_d, call=call_wn, colf=colf_wn_t,
                     recip=recip_wn_t, xt=xta_t, wp_self=wt["wp_self_a"],
                     wp_rel=wt["wp_rel_wn"], b_self=bt["b_self_a_rep"],
                     bf_self=bt["bf_self_a_rep"], out=oa),
            ]

            def emit_window(w, r):
                call = r["call"]
                g = gpool.tile([P, call * D], fp8, tag="g")
                nc.sync.dma_start(g[:], r["gdram"][w * P:(w + 1) * P, :])

                oh = ohpool.tile([P, call, P], fp8, tag="oh")
                nc.vector.tensor_tensor(
                    out=oh[:],
                    in0=r["colf"][:, w * call:(w + 1) * call, None]
                        .to_broadcast([P, call, P]),
                    in1=iota_t[:, None, :].to_broadcast([P, call, P]),
                    op=OP.is_equal)

                m_ps = mpool.tile([P, D], f32, tag="m")
                for k in range(call):
                    nc.tensor.matmul(out=m_ps[:], lhsT=oh[:, k, :],
                                     rhs=g[:, k * D:(k + 1) * D],
                                     start=(k == 0), stop=(k == call - 1))

                m_sb = sbpool.tile([P, D], bf16, tag="m_sb")
                nc.vector.tensor_tensor(
                    out=m_sb[:], in0=m_ps[:],
                    in1=r["recip"][:, w:w + 1].to_broadcast([P, D]), op=OP.mult)

                mt = []
                for h2 in range(2):
                    t_ps = tpool.tile([P, P], bf16, tag="t")
                    nc.tensor.transpose(out=t_ps[:],
                                        in_=m_sb[:, h2 * P:(h2 + 1) * P],
                                        identity=ident_t[:])
                    mt_sb = sbpool.tile([P, P], bf16, tag=f"mt{h2}")
                    nc.vector.tensor_copy(out=mt_sb[:], in_=t_ps[:])
                    mt.append(mt_sb)

                rel_ps = dpool.tile([P, 2 * D], f32, tag="rel")
                nc.tensor.matmul(out=rel_ps[:], lhsT=mt[0][:],
                                 rhs=r["wp_rel"][0][:], start=True, stop=False)
                nc.tensor.matmul(out=rel_ps[:], lhsT=mt[1][:],
                                 rhs=r["wp_rel"][1][:], start=False, stop=True)

                self_ps = dpool.tile([P, 2 * D], f32, tag="self")
                xsl0 = r["xt"][0][:, w * P:(w + 1) * P]
                xsl1 = r["xt"][1][:, w * P:(w + 1) * P]
                nc.tensor.matmul(out=self_ps[:], lhsT=xsl0,
                                 rhs=r["wp_self"][0][:], start=True, stop=False)
                nc.tensor.matmul(out=self_ps[:], lhsT=xsl1,
                                 rhs=r["wp_self"][1][:], start=False, stop=True)

                def score(zslice, brep, stag):
                    targ = sbpool.tile([P, D], f32, tag=f"targ{stag}")
                    nc.vector.tensor_add(out=targ[:], in0=zslice, in1=brep[:])
                    ttan = sbpool.tile([P, D], f32, tag=f"ttan{stag}")
                    nc.scalar.activation(out=ttan[:], in_=targ[:], func=AF.Tanh)
                    scr = sbpool.tile([P, D], f32, tag=f"scr{stag}")
                    nc.vector.tensor_mul(out=scr[:], in0=ttan[:],
                                         in1=bt["w_rep"][:])
                    s = sbpool.tile([P, 1], f32, tag=f"s{stag}")
                    nc.vector.tensor_reduce(out=s[:], in_=scr[:],
                                            axis=mybir.AxisListType.X, op=OP.add)
                    return s

                s_agg = score(rel_ps[:, D:2 * D], bt["bsem_rep"], "a")
                s_h = score(self_ps[:, D:2 * D], r["bf_self"], "h")

                h_sb = sbpool.tile([P, D], f32, tag="h_sb")
                nc.vector.tensor_add(out=h_sb[:], in0=self_ps[:, 0:D],
                                     in1=r["b_self"][:])

                dsc = sbpool.tile([P, 1], f32, tag="dsc")
                nc.vector.tensor_sub(out=dsc[:], in0=s_h[:], in1=s_agg[:])
                a0 = sbpool.tile([P, 1], f32, tag="a0")
                nc.scalar.activation(out=a0[:], in_=dsc[:], func=AF.Sigmoid)

                diff = sbpool.tile([P, D], f32, tag="diff")
                nc.vector.tensor_sub(out=diff[:], in0=h_sb[:],
                                     in1=rel_ps[:, 0:D])
                outt = sbpool.tile([P, D], bf16, tag="outt")
                nc.vector.scalar_tensor_tensor(
                    out=outt[:], in0=diff[:], scalar=a0[:, 0:1],
                    in1=rel_ps[:, 0:D], op0=OP.mult, op1=OP.add)
                nc.sync.dma_start(r["out"][w * P:(w + 1) * P, :], outt[:])

            for w in range(NWIN):
                for r in rels:
                    emit_window(w, r)

    nc.compile()
    return nc


# ---------------------------------------------------------------- driver
_PROG_CACHE = {}


def _get_program(key):
    if key not in _PROG_CACHE:
        _PROG_CACHE[key] = build_program(*key)
    return _PROG_CACHE[key]


def _make_in_maps(pr):
    shared = dict(
        iota=pr["iota"], ident=pr["ident"],
        bsem_rep=pr["bsem_rep"], w_rep=pr["w_rep"],
        b_self_p_rep=pr["b_self_p_rep"], b_self_a_rep=pr["b_self_a_rep"],
        bf_self_p_rep=pr["bf_self_p_rep"], bf_self_a_rep=pr["bf_self_a_rep"],
        wp_self_p=pr["wp_self_p"], wp_self_a=pr["wp_self_a"],
        wp_rel_wr=pr["wp_rel_wr"], wp_rel_wn=pr["wp_rel_wn"],
    )
    wr, wn = pr["wr"], pr["wn"]
    in_maps = []
    for c in range(NCORES):
        w0, w1 = c * NWIN, (c + 1) * NWIN
        m = dict(shared)
        m["g_wr"] = np.ascontiguousarray(
            wr["g"][w0:w1]).reshape(NWIN * P, wr["call"] * D)
        m["g_wn"] = np.ascontiguousarray(
            wn["g"][w0:w1]).reshape(NWIN * P, wn["call"] * D)
        m["xtp"] = pr["xtp"][c]
        m["xta"] = pr["xta"][c]
        m["colf_wr"] = np.ascontiguousarray(
            wr["colf"][:, w0:w1].reshape(P, -1))
        m["colf_wn"] = np.ascontiguousarray(
            wn["colf"][:, w0:w1].reshape(P, -1))
        m["recip_wr"] = np.ascontiguousarray(wr["recip"][:, w0:w1])
        m["recip_wn"] = np.ascontiguousarray(wn["recip"][:, w0:w1])
        in_maps.append(m)
    return in_maps


def _unpermute(res_list, key, pr):
    node_at = pr[key]["node_at"]          # [NW, P]
    cat = np.concatenate([r for r in res_list], axis=0)  # [NW*P, D]
    out = np.empty((N, D), dtype=F32)
    flat = node_at.ravel()
    v = flat >= 0
    out[flat[v]] = cat[v].astype(F32)
    return out


def run(trace=False, tmpdir=None, **inputs):
    pr = _host_prep(inputs)
    nc = _get_program((pr["wr"]["call"], pr["wn"]["call"]))
    in_maps = _make_in_maps(pr)
    res = run_bass_kernel_spmd(nc, in_maps, list(range(NCORES)),
                               trace=trace, tmpdir=tmpdir)
    op = _unpermute([res.results[c]["op"] for c in range(NCORES)], "wr", pr)
    oa = _unpermute([res.results[c]["oa"] for c in range(NCORES)], "wn", pr)
    return (oa, op), res


def kernel(**inputs):
    (oa, op), _ = run(trace=False, **inputs)
    return (oa, op)


# revision 12
# speedup vs baseline: 1.2463x; 1.0195x over previous
"""HANConv Trainium2 kernel (8 NeuronCores, SPMD, full-I/O contract).

Strategy
--------
Destination-sharded, fully core-independent, zero on-device gather:
  * Destination nodes of each type are PERMUTED into 392 balanced windows
    (128 nodes each) so every window receives <= 2048 edges => exactly
    `call` 128-edge chunks per window, identical across cores (SPMD).
  * Source features for each relation are PRE-GATHERED on host into edge
    order (sorted by destination window) and streamed to each core as
    large contiguous HWDGE DMAs (1 MB per window) -- this replaces the
    gpsimd dma_gather, which costs ~28 us fixed per call on HW.
  * Per window, segment-sum is one-hot matmuls accumulating in PSUM
    (aggregating RAW features; relation + semantic-score transforms fold
    into dense matmuls afterwards with host-folded [W | W@W_sem] pairs).
  * Self path from host-transposed permuted x slices.
  * 2-candidate semantic softmax == sigmoid of score difference.
"""

import heapq
import sys

sys.path.insert(0, "/opt/trn_rl_repo")

import numpy as np
import ml_dtypes

import concourse.bacc as bacc
import concourse.mybir as mybir
import concourse.tile as tile
from concourse.bass_utils import run_bass_kernel_spmd

P = 128
N = 50000
D = 256
NCORES = 8
NWIN = 49                 # windows per core
NW = NWIN * NCORES        # 392 windows total
NPAD = NWIN * P           # 6272 rows per core

BF16 = ml_dtypes.bfloat16
FP8 = ml_dtypes.float8_e4m3
F32 = np.float32


# ---------------------------------------------------------------- host prep
def _balance_windows(deg):
    """LPT-pack destination nodes into NW windows of <=P nodes, balancing
    edge counts. Returns node_at [NW, P] (node id or -1) and call."""
    order = np.argsort(-deg, kind="stable")
    heap = [(0, w, 0) for w in range(NW)]
    heapq.heapify(heap)
    node_at = np.full((NW, P), -1, dtype=np.int64)
    sums = np.zeros(NW, dtype=np.int64)
    for n in order:
        s, w, c = heapq.heappop(heap)
        node_at[w, c] = n
        s += int(deg[n])
        c += 1
        sums[w] = s
        if c < P:
            heapq.heappush(heap, (s, w, c))
    call = max(1, int(-(-sums.max() // P)))
    return node_at, call


def _prep_relation(x_src_b, row, col):
    """Sort edges by balanced dst window; pre-gather source features.

    Returns dict with g [NW, P, call*D] bf16, colf [P, NW, call] bf16,
    recip [P, NW] f32, node_at [NW, P], call.
    """
    E = row.shape[0]
    deg = np.bincount(col, minlength=N)
    node_at, call = _balance_windows(deg)

    # node -> (window, slot)
    win_of = np.empty(N, dtype=np.int64)
    slot_of = np.empty(N, dtype=np.int64)
    flat = node_at.ravel()
    valid = flat >= 0
    pos = np.arange(NW * P, dtype=np.int64)[valid]
    win_of[flat[valid]] = pos // P
    slot_of[flat[valid]] = pos % P

    ew = win_of[col]                       # edge -> window
    order = np.argsort(ew, kind="stable")
    ew_s = ew[order]
    rows_s = row[order].astype(np.int64)
    slots_s = slot_of[col[order]]

    cnt = np.bincount(ew_s, minlength=NW)
    assert cnt.max() <= call * P, (cnt.max(), call * P)
    starts = np.zeros(NW + 1, dtype=np.int64)
    np.cumsum(cnt, out=starts[1:])
    rank = np.arange(E, dtype=np.int64) - starts[ew_s]
    dstpos = ew_s * (call * P) + rank

    rows_pad = np.zeros(NW * call * P, dtype=np.int64)
    slot_pad = np.full(NW * call * P, -1.0, dtype=F32)
    rows_pad[dstpos] = rows_s
    slot_pad[dstpos] = slots_s.astype(F32)

    # pre-gather + per-window [P, call*D] layout (partition = edge-in-chunk)
    g = x_src_b[rows_pad].astype(FP8)                        # [NW*call*P, D]
    g = g.reshape(NW, call, P, D).transpose(0, 2, 1, 3)      # [NW, P, call, D]

    colf = slot_pad.reshape(NW, call, P).transpose(2, 0, 1)  # [P, NW, call]

    recip = np.ones((NW, P), dtype=F32)
    recip[node_at >= 0] = 1.0 / np.maximum(deg[node_at[node_at >= 0]], 1.0)

    return dict(g=g, colf=colf.astype(BF16), recip=recip.T.copy(),
                node_at=node_at, call=call)


def _host_prep(inp):
    xa = np.asarray(inp["x_author"], dtype=F32)
    xp = np.asarray(inp["x_paper"], dtype=F32)
    xa_b = xa.astype(BF16)
    xp_b = xp.astype(BF16)

    pr = {}
    # writes: author -> paper (dst type paper); written: paper -> author
    pr["wr"] = _prep_relation(xa_b, np.asarray(inp["row_writes"]),
                              np.asarray(inp["col_writes"]))
    pr["wn"] = _prep_relation(xp_b, np.asarray(inp["row_written"]),
                              np.asarray(inp["col_written"]))

    # permuted self-path features (dst of wr = paper, dst of wn = author)
    def xperm(x, node_at):
        out = np.zeros((NW * P, D), dtype=BF16)
        flat = node_at.ravel()
        v = flat >= 0
        out[v] = x[flat[v]]
        return out

    xp_perm = xperm(xp_b, pr["wr"]["node_at"])
    xa_perm = xperm(xa_b, pr["wn"]["node_at"])
    pr["xtp"] = [np.ascontiguousarray(xp_perm[c * NPAD:(c + 1) * NPAD].T)
                 for c in range(NCORES)]
    pr["xta"] = [np.ascontiguousarray(xa_perm[c * NPAD:(c + 1) * NPAD].T)
                 for c in range(NCORES)]

    W_sem = np.asarray(inp["W_sem"], dtype=F32)
    b_sem = np.asarray(inp["b_sem"], dtype=F32)
    w_score = np.asarray(inp["w_score"], dtype=F32)

    def w(name):
        return np.asarray(inp[name], dtype=F32)

    def pair(W):
        return np.ascontiguousarray(
            np.concatenate([W, W @ W_sem], axis=1)).astype(BF16)

    pr["wp_self_p"] = pair(w("W_self_paper"))
    pr["wp_self_a"] = pair(w("W_self_author"))
    pr["wp_rel_wr"] = pair(w("W_rel_writes"))
    pr["wp_rel_wn"] = pair(w("W_rel_written"))

    rep = lambda v: np.tile(v.astype(F32), (P, 1))
    pr["b_self_p_rep"] = rep(w("b_self_paper"))
    pr["b_self_a_rep"] = rep(w("b_self_author"))
    pr["bf_self_p_rep"] = rep(w("b_self_paper") @ W_sem + b_sem)
    pr["bf_self_a_rep"] = rep(w("b_self_author") @ W_sem + b_sem)
    pr["bsem_rep"] = rep(b_sem)
    pr["w_rep"] = rep(w_score)

    pr["iota"] = np.tile(np.arange(P, dtype=F32), (P, 1)).astype(BF16)
    pr["ident"] = np.eye(P, dtype=F32).astype(BF16)
    return pr


# ---------------------------------------------------------------- program
def build_program(call_wr, call_wn):
    f32 = mybir.dt.float32
    bf16 = mybir.dt.bfloat16
    fp8 = mybir.dt.float8e4
    AF = mybir.ActivationFunctionType
    OP = mybir.AluOpType

    nc = bacc.Bacc("TRN2", target_bir_lowering=False, debug=False)

    g_wr_d = nc.dram_tensor("g_wr", [NWIN * P, call_wr * D], fp8,
                            kind="ExternalInput")
    g_wn_d = nc.dram_tensor("g_wn", [NWIN * P, call_wn * D], fp8,
                            kind="ExternalInput")
    xtp_d = nc.dram_tensor("xtp", [D, NPAD], bf16, kind="ExternalInput")
    xta_d = nc.dram_tensor("xta", [D, NPAD], bf16, kind="ExternalInput")

    wnames = ["wp_self_p", "wp_self_a", "wp_rel_wr", "wp_rel_wn"]
    wdram = {n: nc.dram_tensor(n, [D, 2 * D], bf16, kind="ExternalInput")
             for n in wnames}
    bnames = ["b_self_p_rep", "b_self_a_rep", "bf_self_p_rep",
              "bf_self_a_rep", "bsem_rep", "w_rep"]
    bdram = {n: nc.dram_tensor(n, [P, D], f32, kind="ExternalInput")
             for n in bnames}
    iota_d = nc.dram_tensor("iota", [P, P], bf16, kind="ExternalInput")
    ident_d = nc.dram_tensor("ident", [P, P], bf16, kind="ExternalInput")

    colf_wr_d = nc.dram_tensor("colf_wr", [P, NWIN * call_wr], bf16,
                               kind="ExternalInput")
    colf_wn_d = nc.dram_tensor("colf_wn", [P, NWIN * call_wn], bf16,
                               kind="ExternalInput")
    recip_wr_d = nc.dram_tensor("recip_wr", [P, NWIN], f32, kind="ExternalInput")
    recip_wn_d = nc.dram_tensor("recip_wn", [P, NWIN], f32, kind="ExternalInput")

    oa = nc.dram_tensor("oa", [NPAD, D], bf16, kind="ExternalOutput")
    op_ = nc.dram_tensor("op", [NPAD, D], bf16, kind="ExternalOutput")

    with tile.TileContext(nc) as tc:
        with tc.tile_pool(name="const", bufs=1) as cpool, \
             tc.tile_pool(name="g", bufs=3) as gpool, \
             tc.tile_pool(name="oh", bufs=3) as ohpool, \
             tc.tile_pool(name="sb", bufs=4) as sbpool, \
             tc.tile_pool(name="mps", bufs=2, space="PSUM") as mpool, \
             tc.tile_pool(name="tps", bufs=2, space="PSUM") as tpool, \
             tc.tile_pool(name="dps", bufs=2, space="PSUM") as dpool:

            def load(dram, shape, dtype, tag):
                t = cpool.tile(shape, dtype, tag=tag)
                nc.sync.dma_start(t[:], dram)
                return t

            iota_t = load(iota_d[:], [P, P], bf16, "c_iota")
            ident_t = load(ident_d[:], [P, P], bf16, "c_ident")
            wt = {n: (load(wdram[n][0:P, :], [P, 2 * D], bf16, f"c_{n}0"),
                      load(wdram[n][P:D, :], [P, 2 * D], bf16, f"c_{n}1"))
                  for n in wnames}
            bt = {n: load(bdram[n][:], [P, D], f32, f"c_{n}") for n in bnames}
            xtp_t = (load(xtp_d[0:P, :], [P, NPAD], bf16, "c_xtp0"),
                     load(xtp_d[P:D, :], [P, NPAD], bf16, "c_xtp1"))
            xta_t = (load(xta_d[0:P, :], [P, NPAD], bf16, "c_xta0"),
                     load(xta_d[P:D, :], [P, NPAD], bf16, "c_xta1"))
            colf_wr_t = load(colf_wr_d[:], [P, NWIN * call_wr], bf16, "c_colfwr")
            colf_wn_t = load(colf_wn_d[:], [P, NWIN * call_wn], bf16, "c_colfwn")
            recip_wr_t = load(recip_wr_d[:], [P, NWIN], f32, "c_recipwr")
            recip_wn_t = load(recip_wn_d[:], [P, NWIN], f32, "c_recipwn")

            rels = [
                dict(tag="wr", gdram=g_wr_d, call=call_wr, colf=colf_wr_t,
                     recip=recip_wr_t, xt=xtp_t, wp_self=wt["wp_self_p"],
                     wp_rel=wt["wp_rel_wr"], b_self=bt["b_self_p_rep"],
                     bf_self=bt["bf_self_p_rep"], out=op_),
                dict(tag="wn", gdram=g_wn_d, call=call_wn, colf=colf_wn_t,
                     recip=recip_wn_t, xt=xta_t, wp_self=wt["wp_self_a"],
                     wp_rel=wt["wp_rel_wn"], b_self=bt["b_self_a_rep"],
                     bf_self=bt["bf_self_a_rep"], out=oa),
            ]

            def emit_window(w, r, ri):
                # split DMA load across the two HWDGE rings (SP via nc.sync,
                # ACT via nc.scalar): g loads alternate rings per
                # window-relation; the output store takes the other ring.
                e_g = nc.sync if (2 * w + ri) % 2 == 0 else nc.scalar
                e_out = nc.scalar if (2 * w + ri) % 2 == 0 else nc.sync
                call = r["call"]
                g = gpool.tile([P, call * D], fp8, tag="g")
                e_g.dma_start(g[:], r["gdram"][w * P:(w + 1) * P, :])

                oh = ohpool.tile([P, call, P], fp8, tag="oh")
                nc.vector.tensor_tensor(
                    out=oh[:],
                    in0=r["colf"][:, w * call:(w + 1) * call, None]
                        .to_broadcast([P, call, P]),
                    in1=iota_t[:, None, :].to_broadcast([P, call, P]),
                    op=OP.is_equal)

                m_ps = mpool.tile([P, D], f32, tag="m")
                for k in range(call):
                    nc.tensor.matmul(out=m_ps[:], lhsT=oh[:, k, :],
                                     rhs=g[:, k * D:(k + 1) * D],
                                     start=(k == 0), stop=(k == call - 1))

                m_sb = sbpool.tile([P, D], bf16, tag="m_sb")
                nc.vector.tensor_tensor(
                    out=m_sb[:], in0=m_ps[:],
                    in1=r["recip"][:, w:w + 1].to_broadcast([P, D]), op=OP.mult)

                mt = []
                for h2 in range(2):
                    t_ps = tpool.tile([P, P], bf16, tag="t")
                    nc.tensor.transpose(out=t_ps[:],
                                        in_=m_sb[:, h2 * P:(h2 + 1) * P],
                                        identity=ident_t[:])
                    mt_sb = sbpool.tile([P, P], bf16, tag=f"mt{h2}")
                    nc.vector.tensor_copy(out=mt_sb[:], in_=t_ps[:])
                    mt.append(mt_sb)

                rel_ps = dpool.tile([P, 2 * D], f32, tag="rel")
                nc.tensor.matmul(out=rel_ps[:], lhsT=mt[0][:],
                                 rhs=r["wp_rel"][0][:], start=True, stop=False)
                nc.tensor.matmul(out=rel_ps[:], lhsT=mt[1][:],
                                 rhs=r["wp_rel"][1][:], start=False, stop=True)

                self_ps = dpool.tile([P, 2 * D], f32, tag="self")
                xsl0 = r["xt"][0][:, w * P:(w + 1) * P]
                xsl1 = r["xt"][1][:, w * P:(w + 1) * P]
                nc.tensor.matmul(out=self_ps[:], lhsT=xsl0,
                                 rhs=r["wp_self"][0][:], start=True, stop=False)
                nc.tensor.matmul(out=self_ps[:], lhsT=xsl1,
                                 rhs=r["wp_self"][1][:], start=False, stop=True)

                def score(zslice, brep, stag):
                    targ = sbpool.tile([P, D], f32, tag=f"targ{stag}")
                    nc.vector.tensor_add(out=targ[:], in0=zslice, in1=brep[:])
                    ttan = sbpool.tile([P, D], f32, tag=f"ttan{stag}")
                    nc.scalar.activation(out=ttan[:], in_=targ[:], func=AF.Tanh)
                    scr = sbpool.tile([P, D], f32, tag=f"scr{stag}")
                    nc.vector.tensor_mul(out=scr[:], in0=ttan[:],
                                         in1=bt["w_rep"][:])
                    s = sbpool.tile([P, 1], f32, tag=f"s{stag}")
                    nc.vector.tensor_reduce(out=s[:], in_=scr[:],
                                            axis=mybir.AxisListType.X, op=OP.add)
                    return s

                s_agg = score(rel_ps[:, D:2 * D], bt["bsem_rep"], "a")
                s_h = score(self_ps[:, D:2 * D], r["bf_self"], "h")

                h_sb = sbpool.tile([P, D], f32, tag="h_sb")
                nc.vector.tensor_add(out=h_sb[:], in0=self_ps[:, 0:D],
                                     in1=r["b_self"][:])

                dsc = sbpool.tile([P, 1], f32, tag="dsc")
                nc.vector.tensor_sub(out=dsc[:], in0=s_h[:], in1=s_agg[:])
                a0 = sbpool.tile([P, 1], f32, tag="a0")
                nc.scalar.activation(out=a0[:], in_=dsc[:], func=AF.Sigmoid)

                diff = sbpool.tile([P, D], f32, tag="diff")
                nc.vector.tensor_sub(out=diff[:], in0=h_sb[:],
                                     in1=rel_ps[:, 0:D])
                outt = sbpool.tile([P, D], bf16, tag="outt")
                nc.vector.scalar_tensor_tensor(
                    out=outt[:], in0=diff[:], scalar=a0[:, 0:1],
                    in1=rel_ps[:, 0:D], op0=OP.mult, op1=OP.add)
                e_out.dma_start(r["out"][w * P:(w + 1) * P, :], outt[:])

            for w in range(NWIN):
                for ri, r in enumerate(rels):
                    emit_window(w, r, ri)

    nc.compile()
    return nc


# ---------------------------------------------------------------- driver
_PROG_CACHE = {}


def _get_program(key):
    if key not in _PROG_CACHE:
        _PROG_CACHE[key] = build_program(*key)
    return _PROG_CACHE[key]


def _make_in_maps(pr):
    shared = dict(
        iota=pr["iota"], ident=pr["ident"],
        bsem_rep=pr["bsem_rep"], w_rep=pr["w_rep"],
        b_self_p_rep=pr["b_self_p_rep"], b_self_a_rep=pr["b_self_a_rep"],
        bf_self_p_rep=pr["bf_self_p_rep"], bf_self_a_rep=pr["bf_self_a_rep"],
        wp_self_p=pr["wp_self_p"], wp_self_a=pr["wp_self_a"],
        wp_rel_wr=pr["wp_rel_wr"], wp_rel_wn=pr["wp_rel_wn"],
    )
    wr, wn = pr["wr"], pr["wn"]
    in_maps = []
    for c in range(NCORES):
        w0, w1 = c * NWIN, (c + 1) * NWIN
        m = dict(shared)
        m["g_wr"] = np.ascontiguousarray(
            wr["g"][w0:w1]).reshape(NWIN * P, wr["call"] * D)
        m["g_wn"] = np.ascontiguousarray(
            wn["g"][w0:w1]).reshape(NWIN * P, wn["call"] * D)
        m["xtp"] = pr["xtp"][c]
        m["xta"] = pr["xta"][c]
        m["colf_wr"] = np.ascontiguousarray(
            wr["colf"][:, w0:w1].reshape(P, -1))
        m["colf_wn"] = np.ascontiguousarray(
            wn["colf"][:, w0:w1].reshape(P, -1))
        m["recip_wr"] = np.ascontiguousarray(wr["recip"][:, w0:w1])
        m["recip_wn"] = np.ascontiguousarray(wn["recip"][:, w0:w1])
        in_maps.append(m)
    return in_maps


def _unpermute(res_list, key, pr):
    node_at = pr[key]["node_at"]          # [NW, P]
    cat = np.concatenate([r for r in res_list], axis=0)  # [NW*P, D]
    out = np.empty((N, D), dtype=F32)
    flat = node_at.ravel()
    v = flat >= 0
    out[flat[v]] = cat[v].astype(F32)
    return out


def run(trace=False, tmpdir=None, **inputs):
    pr = _host_prep(inputs)
    nc = _get_program((pr["wr"]["call"], pr["wn"]["call"]))
    in_maps = _make_in_maps(pr)
    res = run_bass_kernel_spmd(nc, in_maps, list(range(NCORES)),
                               trace=trace, tmpdir=tmpdir)
    op = _unpermute([res.results[c]["op"] for c in range(NCORES)], "wr", pr)
    oa = _unpermute([res.results[c]["oa"] for c in range(NCORES)], "wn", pr)
    return (oa, op), res


def kernel(**inputs):
    (oa, op), _ = run(trace=False, **inputs)
    return (oa, op)
